# revision 1
# baseline (speedup 1.0000x reference)
"""AttentionBlock (GroupNorm + MHSA + proj + residual) on 8 TRN2 NeuronCores.

Sharding: data-parallel over batch (B=8 -> 1 batch element per core), SPMD —
one Bass program, per-core input maps.

Per-core math (C=512, T=1024, 8 heads, ch=64, 32 groups):
  h   = GroupNorm(x)                          (512, 1024)
  qkv = Wqkv h + b   (q,k pre-scaled by 64^-0.25 on host, folded into W,b)
  per head:  S^T(s,t) = k^T q                 (no max-subtraction: |S|<~8)
             P^T = exp(S^T)
             a   = v P^T  (with an extra all-ones column in v^T giving Z row)
             a  /= Z  (broadcast via selector matmul)
  out = Wproj a + b_proj;  y = x + out

Matmuls run as float32r (full-rate fp32 on the PE at N>=256); tiny GroupNorm
statistic matmuls run as exact fp32.
"""

import sys
import numpy as np

sys.path.insert(0, "/opt/trn_rl_repo")

import concourse.bacc as bacc
import concourse.bass as bass
import concourse.mybir as mybir
import concourse.tile as tile
from concourse import bass_utils

F32 = mybir.dt.float32
F32R = mybir.dt.float32r
BF16 = mybir.dt.bfloat16
AF = mybir.ActivationFunctionType
ALU = mybir.AluOpType

B, C, HH, WW = 8, 512, 32, 32
T = HH * WW            # 1024
NH = 8                 # heads
CH = C // NH           # 64 per-head dim
NCT = C // 128         # 4 channel tiles
NTT = T // 128         # 8 seq tiles
SCALE = 1.0 / np.sqrt(np.sqrt(CH))
EPS = 1e-5

_CACHE = {}


def build_kernel(debug=False):
    nc = bacc.Bacc(
        "TRN2", target_bir_lowering=False, debug=debug, num_devices=8
    )

    x_d = nc.dram_tensor("x", (C, T), F32, kind="ExternalInput")
    wqkvT_d = nc.dram_tensor("wqkvT", (C, 3 * C + 8), F32R, kind="ExternalInput")
    wprojT_d = nc.dram_tensor("wprojT", (C, C), F32R, kind="ExternalInput")
    # cblob: [gamma | beta | gred] per-128-partition constants
    cblob_d = nc.dram_tensor("cblob", (128, 2 * NCT + 8), F32, kind="ExternalInput")
    # rblob: [bqkv (3C) | bproj (C) | ones (512)] single-partition rows
    rblob_d = nc.dram_tensor("rblob", (1, 3 * C + 8 + C + 512), F32R, kind="ExternalInput")
    gbcast_d = nc.dram_tensor("gbcast", (8, 128), F32, kind="ExternalInput")
    e8_d = nc.dram_tensor("e8", (8, NH * CH), F32R, kind="ExternalInput")
    y_d = nc.dram_tensor("y", (C, T), F32, kind="ExternalOutput")

    with tile.TileContext(nc) as tc:
        with (
            tc.tile_pool(name="single", bufs=1) as single,
            tc.tile_pool(name="xp", bufs=NCT) as xp,
            tc.tile_pool(name="wq", bufs=NCT) as wqp,
            tc.tile_pool(name="wp", bufs=NCT) as wpp,
            tc.tile_pool(name="big", bufs=NCT) as bigp,      # h tiles then a_all
            tc.tile_pool(name="vt", bufs=NTT) as vtp,
            tc.tile_pool(name="qk", bufs=2) as qkp,
            tc.tile_pool(name="pt", bufs=3) as ptp,
            tc.tile_pool(name="aun", bufs=1) as aunp,
            tc.tile_pool(name="zp", bufs=1) as zp,
            tc.tile_pool(name="tmp", bufs=2) as tmpp,
            tc.tile_pool(name="gn", bufs=8) as gnp,
            tc.tile_pool(name="pA", bufs=2, space="PSUM") as ppA,
            tc.tile_pool(name="pB", bufs=2, space="PSUM") as ppB,
        ):
            # ---------------- constant / weight loads ----------------
            cblob = single.tile([128, 2 * NCT + 8], F32, tag="cblob")
            nc.sync.dma_start(out=cblob[:, :], in_=cblob_d.ap())
            gamma = cblob[:, 0:NCT]
            beta = cblob[:, NCT:2 * NCT]
            gred = cblob[:, 2 * NCT:2 * NCT + 8]
            gbcast = single.tile([8, 128], F32, tag="gbcast")
            nc.sync.dma_start(out=gbcast[:, :], in_=gbcast_d.ap())
            e8 = single.tile([8, NH * CH], F32R, tag="e8")
            nc.sync.dma_start(out=e8[:, :], in_=e8_d.ap())
            NQKV = 3 * C + 8
            rblob = single.tile([1, NQKV + C + 512], F32R, tag="rblob")
            nc.sync.dma_start(out=rblob[:, :], in_=rblob_d.ap())
            bqkv = rblob[:, 0:NQKV]
            bproj = rblob[:, NQKV:NQKV + C]
            ones = rblob[:, NQKV + C:NQKV + C + 512]

            wqbig = single.tile([128, NCT, 3 * C + 8], F32R, tag="wqbig")
            nc.sync.dma_start(
                out=wqbig[:, :, :],
                in_=wqkvT_d.ap().rearrange("(c p) t -> p c t", p=128),
            )
            wq_t = [wqbig[:, ct, :] for ct in range(NCT)]
            wpbig = single.tile([128, NCT, C], F32R, tag="wpbig")
            nc.sync.dma_start(
                out=wpbig[:, :, :],
                in_=wprojT_d.ap().rearrange("(c p) t -> p c t", p=128),
            )
            wp_t = [wpbig[:, ct, :] for ct in range(NCT)]
            xbig = single.tile([128, NCT, T], F32, tag="xbig")
            xr4 = x_d.ap().rearrange("(c p) t -> p c t", p=128)
            for ct in range(NCT):
                nc.sync.dma_start(out=xbig[:, ct, :], in_=xr4[:, ct, :])
            x_t = [xbig[:, ct, :] for ct in range(NCT)]

            # ---------------- GroupNorm ----------------
            # per-channel stats via bn_stats/bn_aggr, cross-channel (16/group)
            # reduction + broadcast via tiny exact-fp32 matmuls.
            cs = gnp.tile([128, 2 * NCT], F32, tag="cs")  # cols ct: mean, NCT+ct: E[x^2]
            for ct in range(NCT):
                xr = x_t[ct][:, :].rearrange("p (n f) -> p n f", f=512)
                st = gnp.tile([128, 2, 6], F32, tag="st")
                for sg in range(2):
                    nc.vector.bn_stats(out=st[:, sg, :], in_=xr[:, sg, :])
                mv = gnp.tile([128, 2], F32, tag="mv")
                nc.vector.bn_aggr(out=mv[:, :], in_=st[:, :, :])
                nc.vector.tensor_copy(out=cs[:, ct:ct + 1], in_=mv[:, 0:1])
                nc.vector.tensor_mul(
                    out=cs[:, NCT + ct:NCT + ct + 1], in0=mv[:, 0:1], in1=mv[:, 0:1]
                )
                nc.vector.tensor_add(
                    out=cs[:, NCT + ct:NCT + ct + 1],
                    in0=cs[:, NCT + ct:NCT + ct + 1],
                    in1=mv[:, 1:2],
                )
            gsp = ppA.tile([8, 2 * NCT], F32, tag="A")
            nc.tensor.matmul(gsp[:, :], gred[:, :], cs[:, :], start=True, stop=True)
            gs = gnp.tile([8, 2 * NCT], F32, tag="gs")
            nc.vector.tensor_copy(out=gs[:, :], in_=gsp[:, :])
            # rstd = 1/sqrt(var+eps) with one Newton refinement
            t1 = gnp.tile([8, NCT], F32, tag="t1")
            veps = gnp.tile([8, NCT], F32, tag="veps")
            nc.vector.tensor_mul(out=t1[:, :], in0=gs[:, 0:NCT], in1=gs[:, 0:NCT])
            nc.vector.tensor_sub(out=veps[:, :], in0=gs[:, NCT:], in1=t1[:, :])
            nc.vector.tensor_scalar_add(out=veps[:, :], in0=veps[:, :], scalar1=EPS)
            sq = gnp.tile([8, NCT], F32, tag="sq")
            nc.scalar.activation(out=sq[:, :], in_=veps[:, :], func=AF.Sqrt)
            r0 = gnp.tile([8, NCT], F32, tag="r0")
            nc.vector.reciprocal(out=r0[:, :], in_=sq[:, :])
            nc.vector.tensor_mul(out=t1[:, :], in0=r0[:, :], in1=r0[:, :])
            nc.vector.tensor_mul(out=t1[:, :], in0=t1[:, :], in1=veps[:, :])
            nc.vector.tensor_scalar(
                out=t1[:, :], in0=t1[:, :], scalar1=-0.5, scalar2=1.5,
                op0=ALU.mult, op1=ALU.add,
            )
            nc.vector.tensor_mul(out=r0[:, :], in0=r0[:, :], in1=t1[:, :])
            mr = gnp.tile([8, 2 * NCT], F32, tag="mr")  # cols 2ct: mean_g, 2ct+1: rstd_g
            for ct in range(NCT):
                nc.vector.tensor_copy(
                    out=mr[:, 2 * ct:2 * ct + 1], in_=gs[:, ct:ct + 1]
                )
                nc.vector.tensor_copy(
                    out=mr[:, 2 * ct + 1:2 * ct + 2], in_=r0[:, ct:ct + 1]
                )
            h_t = []
            for ct in range(NCT):
                mrc = ppB.tile([128, 2], F32, tag="B")
                nc.tensor.matmul(
                    mrc[:, :], gbcast[:, :], mr[:, 2 * ct:2 * ct + 2],
                    start=True, stop=True,
                )
                sc = gnp.tile([128, 1], F32, tag="sc")
                sh = gnp.tile([128, 1], F32, tag="sh")
                nc.vector.tensor_mul(
                    out=sc[:, :], in0=mrc[:, 1:2], in1=gamma[:, ct:ct + 1]
                )
                nc.vector.tensor_mul(out=sh[:, :], in0=mrc[:, 0:1], in1=sc[:, :])
                nc.vector.tensor_sub(
                    out=sh[:, :], in0=beta[:, ct:ct + 1], in1=sh[:, :]
                )
                ht = bigp.tile([128, T], F32R, tag="big")
                nc.vector.tensor_scalar(
                    out=ht[:, :], in0=x_t[ct][:, :], scalar1=sc[:, :],
                    scalar2=sh[:, :], op0=ALU.mult, op1=ALU.add,
                )
                h_t.append(ht)

            # ---------------- v^T (+ ones column) ----------------
            # v^T(t, c_v) for all heads at once; ones column at per-head col 64
            # makes the av matmul also produce the softmax denominator Z.
            # v^T for all heads; the v-section of wqkvT carries one extra
            # zero-weight column per head with bias 1.0, so column 64 of each
            # head block is all-ones -> the av matmul also produces Z.
            VW = NH * (CH + 1)  # 520
            vt_t = []
            for tt in range(NTT):
                vps = ppB.tile([128, VW], F32, tag="B")
                for seg in ((0, 512), (512, VW)):
                    dst = vps[:, seg[0]:seg[1]]
                    for ct in range(NCT):
                        nc.tensor.matmul(
                            dst,
                            h_t[ct][:, tt * 128:(tt + 1) * 128],
                            wq_t[ct][:, 2 * C + seg[0]:2 * C + seg[1]],
                            start=(ct == 0), stop=False,
                        )
                    nc.tensor.matmul(
                        dst, ones[0:1, 0:128],
                        bqkv[0:1, 2 * C + seg[0]:2 * C + seg[1]],
                        start=False, stop=True,
                    )
                vt = vtp.tile([128, VW], BF16, tag="vt")
                nc.vector.tensor_copy(out=vt[:, :], in_=vps[:, :])
                vt_t.append(vt)

            # ---------------- per-pair qkv + attention ----------------
            # One big a_un tile: head h occupies [:, h, :]; row 64 holds the
            # softmax denominator Z so a single DMA can gather all 8 Z rows.
            aunbig = aunp.tile([CH + 1, NH, T], F32, tag="aun")
            zall = zp.tile([8, T], F32, tag="z")
            for p in range(NH // 2):
                # q and k for heads 2p, 2p+1: psum rows 0..63 = even head,
                # 64..127 = odd head -> scores matmuls stay partition-aligned.
                qps = ppA.tile([128, T], F32, tag="A")
                kps = ppB.tile([128, T], F32, tag="B")
                for psum, off in ((qps, 256 * p), (kps, 256 * p + 128)):
                    for nq in range(2):
                        dst = psum[:, nq * 512:(nq + 1) * 512]
                        for ct in range(NCT):
                            nc.tensor.matmul(
                                dst,
                                wq_t[ct][:, off:off + 128],
                                h_t[ct][:, nq * 512:(nq + 1) * 512],
                                start=(ct == 0), stop=False,
                            )
                        nc.tensor.matmul(
                            dst,
                            bqkv[0:1, off:off + 128],
                            ones[0:1, 0:512],
                            start=False, stop=True,
                        )
                qp_s = qkp.tile([128, T], BF16, tag="qpair")
                nc.vector.tensor_copy(out=qp_s[:, :], in_=qps[:, :])
                kp_s = qkp.tile([128, T], BF16, tag="kpair")
                nc.vector.tensor_copy(out=kp_s[:, :], in_=kps[:, :])

                avps = [
                    ppB.tile([CH + 1, T], F32, tag="B", name=f"av{p}_{hl}")
                    for hl in range(2)
                ]
                for st_ in range(NTT):
                    # scores for both heads interleaved: the K=64 matmuls
                    # land on row strips 0/64 and pack the PE concurrently
                    scps = [
                        ppA.tile([128, T], F32, tag="A", name=f"sc{p}_{st_}_{hl}")
                        for hl in range(2)
                    ]
                    for nq in range(2):
                        for hl in range(2):
                            base = 64 * hl
                            nc.tensor.matmul(
                                scps[hl][:, nq * 512:(nq + 1) * 512],
                                kp_s[base:base + 64, st_ * 128:(st_ + 1) * 128],
                                qp_s[base:base + 64, nq * 512:(nq + 1) * 512],
                                start=True, stop=True,
                            )
                    pts = []
                    for hl in range(2):
                        pt = ptp.tile([128, T], BF16, tag="pt", name=f"pt{hl}")
                        nc.scalar.activation(
                            out=pt[:, :], in_=scps[hl][:, :], func=AF.Exp
                        )
                        pts.append(pt)
                    for nq in range(2):
                        for hl in range(2):
                            h_ = 2 * p + hl
                            nc.tensor.matmul(
                                avps[hl][:, nq * 512:(nq + 1) * 512],
                                vt_t[st_][:, h_ * (CH + 1):(h_ + 1) * (CH + 1)],
                                pts[hl][:, nq * 512:(nq + 1) * 512],
                                start=(st_ == 0), stop=(st_ == NTT - 1),
                            )
                for hl in range(2):
                    nc.vector.tensor_copy(
                        out=aunbig[:, 2 * p + hl, :], in_=avps[hl][:, :]
                    )

            # ---------------- softmax normalization ----------------
            nc.sync.dma_start(out=zall[:, :], in_=aunbig[CH:CH + 1, :, :])
            invzf = zp.tile([8, T], F32, tag="invzf")
            zscr = zp.tile([8, T], F32, tag="zscr")
            nc.vector.reciprocal_approx_accurate(
                out=invzf[:, :], in_=zall[:, :], scratch=zscr[:, :]
            )
            invz = zp.tile([8, T], F32R, tag="invz")
            with nc.allow_low_precision(reason="fp32r matmul operand"):
                nc.vector.tensor_copy(out=invz[:, :], in_=invzf[:, :])
            a_all = []
            for ct in range(NCT):
                a_all.append(bigp.tile([128, T], F32R, tag="big", name=f"aall{ct}"))
            for h_ in range(NH):
                zb = ppA.tile([CH, T], F32, tag="A")
                for nq in range(2):
                    nc.tensor.matmul(
                        zb[:, nq * 512:(nq + 1) * 512],
                        e8[:, h_ * CH:(h_ + 1) * CH],
                        invz[:, nq * 512:(nq + 1) * 512],
                        start=True, stop=True,
                    )
                if h_ % 2 == 0:
                    dst = a_all[h_ // 2][0:CH, :]
                    nc.vector.tensor_mul(
                        out=dst, in0=aunbig[0:CH, h_, :], in1=zb[:, :]
                    )
                else:
                    atmp = tmpp.tile([CH, T], F32R, tag="atmp")
                    nc.vector.tensor_mul(
                        out=atmp[:, :], in0=aunbig[0:CH, h_, :], in1=zb[:, :]
                    )
                    nc.sync.dma_start(
                        out=a_all[h_ // 2][CH:2 * CH, :], in_=atmp[:, :]
                    )

            # ---------------- out projection + residual ----------------
            for m in range(NCT):
                pps = ppA.tile([128, T], F32, tag="A")
                for nq in range(2):
                    dst = pps[:, nq * 512:(nq + 1) * 512]
                    for ck in range(NCT):
                        nc.tensor.matmul(
                            dst,
                            wp_t[ck][:, m * 128:(m + 1) * 128],
                            a_all[ck][:, nq * 512:(nq + 1) * 512],
                            start=(ck == 0), stop=False,
                        )
                    nc.tensor.matmul(
                        dst,
                        bproj[0:1, m * 128:(m + 1) * 128],
                        ones[0:1, 0:512],
                        start=False, stop=True,
                    )
                nc.vector.tensor_add(
                    out=x_t[m][:, :], in0=pps[:, :], in1=x_t[m][:, :]
                )
                nc.sync.dma_start(
                    out=y_d.ap()[m * 128:(m + 1) * 128, :], in_=x_t[m][:, :]
                )

    nc.compile()
    return nc


def make_in_maps(x, gn_weight, gn_bias, w_qkv, b_qkv, w_proj, b_proj):
    x = np.asarray(x, dtype=np.float32)
    w_qkv = np.asarray(w_qkv, dtype=np.float32)
    b_qkv = np.asarray(b_qkv, dtype=np.float32)
    scale = np.float32(SCALE)
    wq = w_qkv.copy()
    bq = b_qkv.copy()
    for hd in range(NH):
        sl = slice(3 * CH * hd, 3 * CH * hd + 2 * CH)  # q,k rows of this head
        wq[sl] *= scale
        bq[sl] *= scale
    # Column order expected by the kernel: per head-pair p the contiguous
    # blocks [q(2p) | q(2p+1) | k(2p) | k(2p+1)] (256 cols each), then all
    # v blocks. Makes every PE stationary-operand slice a single free dim.
    perm = []
    for p in range(NH // 2):
        for hd in (2 * p, 2 * p + 1):
            perm.extend(range(3 * CH * hd, 3 * CH * hd + CH))          # q
        for hd in (2 * p, 2 * p + 1):
            perm.extend(range(3 * CH * hd + CH, 3 * CH * hd + 2 * CH))  # k
    for hd in range(NH):
        perm.extend(range(3 * CH * hd + 2 * CH, 3 * CH * hd + 3 * CH))  # v
    perm = np.asarray(perm)
    wq = wq[perm]
    bq = bq[perm]
    wprojT = np.ascontiguousarray(np.asarray(w_proj, np.float32).T)  # (C, C)
    gamma = np.asarray(gn_weight, np.float32).reshape(NCT, 128).T
    beta = np.asarray(gn_bias, np.float32).reshape(NCT, 128).T
    gred = np.zeros((128, 8), np.float32)
    gbcast = np.zeros((8, 128), np.float32)
    for c in range(128):
        gred[c, c // 16] = 1.0 / 16.0
        gbcast[c // 16, c] = 1.0
    e8 = np.zeros((8, NH * CH), np.float32)
    for g in range(8):
        e8[g, g * CH:(g + 1) * CH] = 1.0
    cblob = np.ascontiguousarray(
        np.concatenate([gamma, beta, gred], axis=1)
    )                                                        # (128, 16)
    # v-section gains a zero-weight column with bias 1.0 per head (the Z
    # column of v^T); qk section stays 1024 wide.
    wq2 = np.zeros((C, 3 * C + 8), np.float32)
    bq2 = np.zeros(3 * C + 8, np.float32)
    wq2[:, 0:2 * C] = wq.T[:, 0:2 * C]
    bq2[0:2 * C] = bq[0:2 * C]
    for hd in range(NH):
        wq2[:, 2 * C + 65 * hd:2 * C + 65 * hd + CH] = \
            wq.T[:, 2 * C + CH * hd:2 * C + CH * (hd + 1)]
        bq2[2 * C + 65 * hd:2 * C + 65 * hd + CH] = \
            bq[2 * C + CH * hd:2 * C + CH * (hd + 1)]
        bq2[2 * C + 65 * hd + CH] = 1.0
    wqkvT2 = np.ascontiguousarray(wq2)
    rblob = np.concatenate(
        [bq2, np.asarray(b_proj, np.float32), np.ones(512, np.float32)]
    ).reshape(1, -1)

    common = dict(
        wqkvT=wqkvT2, wprojT=wprojT, cblob=cblob, rblob=rblob,
        gbcast=gbcast, e8=e8,
    )
    in_maps = []
    for b in range(B):
        m = dict(common)
        m["x"] = np.ascontiguousarray(x[b].reshape(C, T))
        in_maps.append(m)
    return in_maps


def kernel(x, gn_weight, gn_bias, w_qkv, b_qkv, w_proj, b_proj, _trace=False):
    if "nc" not in _CACHE:
        _CACHE["nc"] = build_kernel()
    nc = _CACHE["nc"]
    in_maps = make_in_maps(x, gn_weight, gn_bias, w_qkv, b_qkv, w_proj, b_proj)
    res = bass_utils.run_bass_kernel_spmd(
        nc, in_maps, core_ids=list(range(B)), trace=_trace
    )
    out = np.stack([r["y"].reshape(C, HH, WW) for r in res.results], axis=0)
    if _trace:
        _CACHE["last_result"] = res
    return out



# revision 5
# speedup vs baseline: 1.8229x; 1.8229x over previous
"""AttentionBlock (GroupNorm + MHSA + proj + residual) on 8 TRN2 NeuronCores.

Sharding: data-parallel over batch (B=8 -> 1 batch element per core), SPMD —
one Bass program, per-core input maps.

Per-core math (C=512, T=1024, 8 heads, ch=64, 32 groups):
  h   = GroupNorm(x)                          (512, 1024)
  qkv = Wqkv h   (q,k pre-scaled by 64^-0.25 on host; q/k biases added on DVE,
                  v bias folded into b_proj on host)
  per head-pair, per t-half (nq):  S^T(s,t) = k^T q   (no max-subtraction)
             P^T = exp(S^T)  (ACT, double-buffered score PSUM)
             a   = v' P^T   (v' has a ones row per head giving the Z row)
             a  /= Z  (broadcast via tiny e2 matmul + DVE mul)
  out = Wproj a + b_proj;  y = x + out

Structure is organized so the ACT engine (exp: 64 x N=1024 instrs ~73us) is
the saturated resource; PE work (~72us) overlaps it, as do DVE/DMA.
PSUM budget (8 banks): scores f32 [128,1024] x2 bufs = 4, av accumulators
[65,512] x2 = 2, aux ring [128,512] x2 = 2.
"""

import sys
import numpy as np

sys.path.insert(0, "/opt/trn_rl_repo")

import concourse.bacc as bacc
import concourse.bass as bass
import concourse.mybir as mybir
import concourse.tile as tile
from concourse import bass_utils

F32 = mybir.dt.float32
F32R = mybir.dt.float32r
BF16 = mybir.dt.bfloat16
AF = mybir.ActivationFunctionType
ALU = mybir.AluOpType

B, C, HH, WW = 8, 512, 32, 32
T = HH * WW            # 1024
NH = 8                 # heads
CH = C // NH           # 64 per-head dim
NCT = C // 128         # 4 channel tiles
NTT = T // 128         # 8 seq tiles
NP = NH // 2           # 4 head pairs
SCALE = 1.0 / np.sqrt(np.sqrt(CH))
EPS = 1e-5
WQK = 2 * C            # 1024 cols of q|k section
WV = C                 # 512 cols of v section (no Z cols in W)
WQ = WQK + WV          # 1536

_CACHE = {}

# cblob column layout (f32, [128, 28]):
#   0:4   gamma per ct
#   4:8   beta per ct
#   8:16  gred (group-reduce 1/16 selector, 8 group-slots)
#   16:24 q/k biases: col 16+2p = q bias of pair p, 17+2p = k bias
#   24:28 bproj (v-bias folded in) per m-tile
CB_GAMMA = 0
CB_BETA = 4
CB_GRED = 8
CB_BQK = 16
CB_BPROJ = 24
CB_W = 28


def build_kernel(debug=False):
    nc = bacc.Bacc(
        "TRN2", target_bir_lowering=False, debug=debug, num_devices=8
    )

    x_d = nc.dram_tensor("x", (C, T), F32, kind="ExternalInput")
    wqkvT_d = nc.dram_tensor("wqkvT", (C, WQ), F32R, kind="ExternalInput")
    wprojT_d = nc.dram_tensor("wprojT", (C, C), F32R, kind="ExternalInput")
    cblob_d = nc.dram_tensor("cblob", (128, CB_W), F32, kind="ExternalInput")
    gbcast_d = nc.dram_tensor("gbcast", (8, 128), F32, kind="ExternalInput")
    e2_d = nc.dram_tensor("e2", (2, 128), F32R, kind="ExternalInput")
    y_d = nc.dram_tensor("y", (C, T), F32, kind="ExternalOutput")

    with tile.TileContext(nc) as tc:
        with (
            tc.tile_pool(name="single", bufs=1) as single,
            tc.tile_pool(name="hp", bufs=1) as hp,
            tc.tile_pool(name="qkp", bufs=1) as qkp,
            tc.tile_pool(name="vtp", bufs=1) as vtp,
            tc.tile_pool(name="ptp", bufs=3) as ptp,
            tc.tile_pool(name="aap", bufs=1) as aap,
            tc.tile_pool(name="gnp", bufs=2) as gnp,
            tc.tile_pool(name="nrm", bufs=2) as nrm,
            tc.tile_pool(name="pp", bufs=1, space="PSUM") as pp,
        ):
            # ---------------- constant / weight / x loads ----------------
            cblob = single.tile([128, CB_W], F32, tag="cblob")
            nc.sync.dma_start(out=cblob[:, :], in_=cblob_d.ap())
            gbcast = single.tile([8, 128], F32, tag="gbcast")
            nc.sync.dma_start(out=gbcast[:, :], in_=gbcast_d.ap())
            e2 = single.tile([2, 128], F32R, tag="e2")
            nc.sync.dma_start(out=e2[:, :], in_=e2_d.ap())

            xbig = single.tile([128, NCT, T], F32, tag="xbig")
            xr4 = x_d.ap().rearrange("(c p) t -> p c t", p=128)
            for ct in range(NCT):
                nc.sync.dma_start(out=xbig[:, ct, :], in_=xr4[:, ct, :])
            x_t = [xbig[:, ct, :] for ct in range(NCT)]

            wqbig = single.tile([128, NCT, WQ], F32R, tag="wqbig")
            nc.sync.dma_start(
                out=wqbig[:, :, :],
                in_=wqkvT_d.ap().rearrange("(c p) t -> p c t", p=128),
            )
            wq_t = [wqbig[:, ct, :] for ct in range(NCT)]
            wpbig = single.tile([128, NCT, C], F32R, tag="wpbig")
            nc.sync.dma_start(
                out=wpbig[:, :, :],
                in_=wprojT_d.ap().rearrange("(c p) t -> p c t", p=128),
            )
            wp_t = [wpbig[:, ct, :] for ct in range(NCT)]

            # Pull the ACT table set in early (ln is first used by GN; Exp is
            # the hot loop).  A 1-elem dummy Log right after cblob arrives gets
            # the table load off the critical path.
            warm = gnp.tile([1, 1], F32, tag="warm")
            nc.scalar.activation(out=warm[:, :], in_=cblob[0:1, 0:1], func=AF.Ln)

            # ---------------- GroupNorm ----------------
            # per-channel stats via bn_stats/bn_aggr; cross-channel (16/group)
            # reduction + broadcast via tiny matmuls through the aux psum ring.
            # cs cols 2ct: mean, 2ct+1: E[x^2]
            cs = gnp.tile([128, 2 * NCT], F32, tag="cs")
            for ct in range(NCT):
                xr = x_t[ct][:, :].rearrange("p (n f) -> p n f", f=512)
                st = gnp.tile([128, 2, 6], F32, tag="st")
                for sg in range(2):
                    nc.vector.bn_stats(out=st[:, sg, :], in_=xr[:, sg, :])
                nc.vector.bn_aggr(out=cs[:, 2 * ct:2 * ct + 2], in_=st[:, :, :])
                # E[x^2] = mean*mean + var  (in place over the var column)
                nc.vector.scalar_tensor_tensor(
                    out=cs[:, 2 * ct + 1:2 * ct + 2],
                    in0=cs[:, 2 * ct:2 * ct + 1],
                    scalar=cs[:, 2 * ct:2 * ct + 1],
                    in1=cs[:, 2 * ct + 1:2 * ct + 2],
                    op0=ALU.mult, op1=ALU.add,
                )
            gsp = pp.tile([8, 2 * NCT], F32, tag="aux", bufs=2, name="gsp")
            nc.tensor.matmul(
                gsp[:, :], cblob[:, CB_GRED:CB_GRED + 8], cs[:, :],
                start=True, stop=True,
            )
            # gs cols 2ct: group mean, 2ct+1: group rstd (after ln/exp)
            gs = gnp.tile([8, 2 * NCT], F32, tag="gs")
            nc.vector.tensor_copy(out=gs[:, :], in_=gsp[:, :])
            t1 = gnp.tile([8, NCT], F32, tag="t1")
            nc.vector.tensor_mul(
                out=t1[:, :], in0=gs[:, 0::2], in1=gs[:, 0::2]
            )
            veps = gnp.tile([8, NCT], F32, tag="veps")
            nc.vector.scalar_tensor_tensor(
                out=veps[:, :], in0=gs[:, 1::2], scalar=EPS, in1=t1[:, :],
                op0=ALU.add, op1=ALU.subtract,
            )
            # rstd = exp(-0.5 * ln(var + eps))
            lnv = gnp.tile([8, NCT], F32, tag="lnv")
            nc.scalar.activation(out=lnv[:, :], in_=veps[:, :], func=AF.Ln)
            nc.scalar.activation(out=gs[:, 1::2], in_=lnv[:, :], func=AF.Exp, scale=-0.5)
            h_t = []
            for ct in range(NCT):
                mrc = pp.tile([128, 2], F32, tag="aux", bufs=2, name=f"mrc{ct}")
                nc.tensor.matmul(
                    mrc[:, :], gbcast[:, :], gs[:, 2 * ct:2 * ct + 2],
                    start=True, stop=True,
                )
                sc = gnp.tile([128, 1], F32, tag="sc")
                sh = gnp.tile([128, 1], F32, tag="sh")
                nc.vector.tensor_mul(
                    out=sc[:, :], in0=mrc[:, 1:2],
                    in1=cblob[:, CB_GAMMA + ct:CB_GAMMA + ct + 1],
                )
                nc.vector.tensor_mul(out=sh[:, :], in0=mrc[:, 0:1], in1=sc[:, :])
                nc.vector.tensor_sub(
                    out=sh[:, :],
                    in0=cblob[:, CB_BETA + ct:CB_BETA + ct + 1], in1=sh[:, :],
                )
                ht = hp.tile([128, T], F32R, tag=f"h{ct}", name=f"h{ct}")
                nc.vector.tensor_scalar(
                    out=ht[:, :], in0=x_t[ct][:, :], scalar1=sc[:, :],
                    scalar2=sh[:, :], op0=ALU.mult, op1=ALU.add,
                )
                h_t.append(ht)

            # ---------------- q/k for all pairs (SBUF, bf16) ----------------
            # W col layout per pair p: [q(2p)|q(2p+1)|k(2p)|k(2p+1)] in
            # cols 256p..256p+256; v section at cols 2C..2C+512.
            def emit_qk(p):
                outs = []
                for i, off in ((0, 256 * p), (1, 256 * p + 128)):
                    dst = qkp.tile(
                        [128, T], BF16, tag=f"qk{p}_{i}", name=f"qk{p}_{i}"
                    )
                    for nq in range(2):
                        ps = pp.tile(
                            [128, 512], F32, tag="aux", bufs=2,
                            name=f"qkp{p}_{i}_{nq}",
                        )
                        for ctk in range(NCT):
                            nc.tensor.matmul(
                                ps[:, :],
                                wq_t[ctk][:, off:off + 128],
                                h_t[ctk][:, nq * 512:(nq + 1) * 512],
                                start=(ctk == 0), stop=(ctk == NCT - 1),
                            )
                        nc.vector.tensor_scalar_add(
                            out=dst[:, nq * 512:(nq + 1) * 512], in0=ps[:, :],
                            scalar1=cblob[:, CB_BQK + 2 * p + i:CB_BQK + 2 * p + i + 1],
                        )
                    outs.append(dst)
                return outs  # [q, k]

            qk_s = {0: emit_qk(0)}

            # ---------------- v^T tiles ----------------
            # vt[tt] layout [128 (t), 8 heads, 65]: cols 0..63 = v channels,
            # col 64 = ones (Z row source) via memset.
            vt_t = []
            for tt in range(NTT):
                vps = pp.tile([128, 512], F32, tag="aux", bufs=2, name=f"vps{tt}")
                for ctk in range(NCT):
                    nc.tensor.matmul(
                        vps[:, :],
                        h_t[ctk][:, tt * 128:(tt + 1) * 128],
                        wq_t[ctk][:, WQK:WQK + WV],
                        start=(ctk == 0), stop=(ctk == NCT - 1),
                    )
                vt = vtp.tile([128, NH, CH + 1], BF16, tag=f"vt{tt}", name=f"vt{tt}")
                nc.vector.tensor_copy(
                    out=vt[:, :, 0:CH],
                    in_=vps[:, :].rearrange("p (h c) -> p h c", h=NH),
                )
                nc.vector.memset(vt[:, :, CH:CH + 1], 1.0)
                vt_t.append(vt)

            # ---------------- attention + per-pair normalize ----------------
            aunbig = single.tile([CH + 1, NH, T], F32, tag="aunbig")
            a_all = [
                aap.tile([128, T], F32R, tag=f"aall{ct}", name=f"aall{ct}")
                for ct in range(NCT)
            ]

            def emit_pair(p):
                qs, ks = qk_s[p]
                for nq in range(2):
                    avps = [
                        pp.tile([CH + 1, 512], F32, tag=f"av{hl}", bufs=1,
                                name=f"av{p}_{nq}_{hl}")
                        for hl in range(2)
                    ]
                    for st_ in range(NTT):
                        scps = pp.tile(
                            [128, T], F32, tag="sc", bufs=2,
                            name=f"sc{p}_{nq}_{st_}",
                        )
                        for hl in range(2):
                            base = 64 * hl
                            nc.tensor.matmul(
                                scps[:, hl * 512:(hl + 1) * 512],
                                ks[base:base + 64, st_ * 128:(st_ + 1) * 128],
                                qs[base:base + 64, nq * 512:(nq + 1) * 512],
                                start=True, stop=True,
                            )
                        pt = ptp.tile([128, T], BF16, tag="pt", name=f"pt{p}_{nq}_{st_}")
                        nc.scalar.activation(
                            out=pt[:, :], in_=scps[:, :], func=AF.Exp
                        )
                        for hl in range(2):
                            h_ = 2 * p + hl
                            nc.tensor.matmul(
                                avps[hl][:, :],
                                vt_t[st_][:, h_, :],
                                pt[:, hl * 512:(hl + 1) * 512],
                                start=(st_ == 0), stop=(st_ == NTT - 1),
                            )
                    for hl in range(2):
                        nc.vector.tensor_copy(
                            out=aunbig[:, 2 * p + hl, nq * 512:(nq + 1) * 512],
                            in_=avps[hl][:, :],
                        )

            def emit_normalize(p):
                # Z rows for pair p -> [2, T] tile, reciprocal, broadcast to
                # 64 partitions per head via e2 matmul, then a = aun * invz.
                zp = nrm.tile([2, T], F32, tag="zp", name=f"zp{p}")
                nc.sync.dma_start(out=zp[:, :], in_=aunbig[CH:CH + 1, 2 * p:2 * p + 2, :])
                zscr = nrm.tile([2, T], F32, tag="zscr", name=f"zscr{p}")
                invzf = nrm.tile([2, T], F32, tag="invzf", name=f"invzf{p}")
                nc.vector.reciprocal_approx_accurate(
                    out=invzf[:, :], in_=zp[:, :], scratch=zscr[:, :]
                )
                invz = nrm.tile([2, T], F32R, tag="invz", name=f"invz{p}")
                with nc.allow_low_precision(reason="fp32r matmul operand"):
                    nc.vector.tensor_copy(out=invz[:, :], in_=invzf[:, :])
                atmp = nrm.tile([CH, T], F32R, tag="atmp", name=f"atmp{p}")
                for hl in range(2):
                    h_ = 2 * p + hl
                    for nq in range(2):
                        zb = pp.tile(
                            [CH, 512], F32, tag="aux", bufs=2,
                            name=f"zb{h_}_{nq}",
                        )
                        nc.tensor.matmul(
                            zb[:, :],
                            e2[:, hl * CH:(hl + 1) * CH],
                            invz[:, nq * 512:(nq + 1) * 512],
                            start=True, stop=True,
                        )
                        dst = (
                            a_all[p][0:CH, nq * 512:(nq + 1) * 512]
                            if hl == 0
                            else atmp[:, nq * 512:(nq + 1) * 512]
                        )
                        nc.vector.tensor_mul(
                            out=dst,
                            in0=aunbig[0:CH, h_, nq * 512:(nq + 1) * 512],
                            in1=zb[:, :],
                        )
                nc.sync.dma_start(
                    out=a_all[p][CH:2 * CH, :], in_=atmp[:, :]
                )

            emit_pair(0)
            qk_s[1] = emit_qk(1)
            emit_pair(1)
            emit_normalize(0)
            qk_s[2] = emit_qk(2)
            emit_pair(2)
            emit_normalize(1)
            qk_s[3] = emit_qk(3)
            emit_pair(3)
            emit_normalize(2)
            emit_normalize(3)

            # ---------------- out projection + residual ----------------
            for m in range(NCT):
                for nq in range(2):
                    pps = pp.tile(
                        [128, 512], F32, tag="aux", bufs=2, name=f"pj{m}_{nq}"
                    )
                    for ck in range(NCT):
                        nc.tensor.matmul(
                            pps[:, :],
                            wp_t[ck][:, m * 128:(m + 1) * 128],
                            a_all[ck][:, nq * 512:(nq + 1) * 512],
                            start=(ck == 0), stop=(ck == NCT - 1),
                        )
                    nc.vector.scalar_tensor_tensor(
                        out=x_t[m][:, nq * 512:(nq + 1) * 512],
                        in0=pps[:, :],
                        scalar=cblob[:, CB_BPROJ + m:CB_BPROJ + m + 1],
                        in1=x_t[m][:, nq * 512:(nq + 1) * 512],
                        op0=ALU.add, op1=ALU.add,
                    )
                nc.sync.dma_start(
                    out=y_d.ap()[m * 128:(m + 1) * 128, :], in_=x_t[m][:, :]
                )

    nc.compile()
    return nc


def make_in_maps(x, gn_weight, gn_bias, w_qkv, b_qkv, w_proj, b_proj):
    x = np.asarray(x, dtype=np.float32)
    w_qkv = np.asarray(w_qkv, dtype=np.float32)
    b_qkv = np.asarray(b_qkv, dtype=np.float32)
    w_proj = np.asarray(w_proj, np.float32)
    b_proj = np.asarray(b_proj, np.float32)
    scale = np.float32(SCALE)
    wq = w_qkv.copy()
    bq = b_qkv.copy()
    for hd in range(NH):
        sl = slice(3 * CH * hd, 3 * CH * hd + 2 * CH)  # q,k rows of this head
        wq[sl] *= scale
        bq[sl] *= scale
    # Column order expected by the kernel: per head-pair p the contiguous
    # blocks [q(2p) | q(2p+1) | k(2p) | k(2p+1)] (256 cols each), then all
    # v blocks (64 per head, no Z columns).
    perm = []
    for p in range(NP):
        for hd in (2 * p, 2 * p + 1):
            perm.extend(range(3 * CH * hd, 3 * CH * hd + CH))          # q
        for hd in (2 * p, 2 * p + 1):
            perm.extend(range(3 * CH * hd + CH, 3 * CH * hd + 2 * CH))  # k
    for hd in range(NH):
        perm.extend(range(3 * CH * hd + 2 * CH, 3 * CH * hd + 3 * CH))  # v
    perm = np.asarray(perm)
    wq = wq[perm]
    bq = bq[perm]
    wqkvT = np.ascontiguousarray(wq.T)  # (C, 1536)

    # v bias folded through the projection: out += Wproj @ b_v
    bv_full = np.empty(C, np.float32)
    for hd in range(NH):
        bv_full[hd * CH:(hd + 1) * CH] = b_qkv[3 * CH * hd + 2 * CH:3 * CH * hd + 3 * CH]
    bproj2 = b_proj + w_proj @ bv_full

    wprojT = np.ascontiguousarray(w_proj.T)  # (C, C)
    gamma = np.asarray(gn_weight, np.float32).reshape(NCT, 128).T
    beta = np.asarray(gn_bias, np.float32).reshape(NCT, 128).T
    gred = np.zeros((128, 8), np.float32)
    gbcast = np.zeros((8, 128), np.float32)
    for c in range(128):
        gred[c, c // 16] = 1.0 / 16.0
        gbcast[c // 16, c] = 1.0
    bqk = np.zeros((128, 8), np.float32)
    for p in range(NP):
        bqk[:, 2 * p] = bq[256 * p:256 * p + 128]
        bqk[:, 2 * p + 1] = bq[256 * p + 128:256 * p + 256]
    bproj_col = bproj2.reshape(NCT, 128).T
    cblob = np.ascontiguousarray(
        np.concatenate([gamma, beta, gred, bqk, bproj_col], axis=1)
    )  # (128, 28)
    e2 = np.zeros((2, 128), np.float32)
    e2[0, 0:CH] = 1.0
    e2[1, CH:2 * CH] = 1.0

    common = dict(
        wqkvT=wqkvT, wprojT=wprojT, cblob=cblob, gbcast=gbcast, e2=e2,
    )
    in_maps = []
    for b in range(B):
        m = dict(common)
        m["x"] = np.ascontiguousarray(x[b].reshape(C, T))
        in_maps.append(m)
    return in_maps


def kernel(x, gn_weight, gn_bias, w_qkv, b_qkv, w_proj, b_proj, _trace=False):
    if "nc" not in _CACHE:
        _CACHE["nc"] = build_kernel()
    nc = _CACHE["nc"]
    in_maps = make_in_maps(x, gn_weight, gn_bias, w_qkv, b_qkv, w_proj, b_proj)
    res = bass_utils.run_bass_kernel_spmd(
        nc, in_maps, core_ids=list(range(B)), trace=_trace
    )
    out = np.stack([r["y"].reshape(C, HH, WW) for r in res.results], axis=0)
    if _trace:
        _CACHE["last_result"] = res
    return out
